# revision 23
# baseline (speedup 1.0000x reference)
"""Trainium2 Bass kernel for a single attention head.

reference computation (fp32):
    q = query @ Wq + bq ; k = key @ Wk + bk ; v = value @ Wv + bv
    out = softmax((q @ k^T) / 8) @ v

Sharding: 8 cores, core c -> (batch b = c//2, query-half h = c%2).
Each core computes attention for its 2048 query rows against the full 4096
keys/values of its batch (key/value rows are read by both cores of a pair;
no cross-core communication).

Per-core dataflow (fp32 in/out, fp32r matmuls, rel err ~5e-4):
  - load activations [rows, 512] naturally (SWDGE cast fp32->fp32r),
    PE-transpose 128x128 blocks, DVE-copy PSUM->SBUF to get X^T [c, rows]
  - projections on PE: lhsT = W [c-chunk, d], rhs = X^T -> Qp^T/Kp^T [64, rows]
    (bias folded into the mandatory PSUM->SBUF copy); Qp^T is duplicated to
    SBUF partitions 64:128 and Kp^T stored dual-half (even j-chunks on
    partitions 0:64, odd on 64:128, via a partition-shift SBUF DMA) so that
    scores matmuls can be row-tiled
  - V is re-transposed to natural [rows, 66]; col 64 is all-ones (host pads
    Wv/bv) so the PV matmul also produces the softmax denominator; col 65 is
    zero padding (fp32r requires even free sizes).
  - scores^T tiles: lhsT = Kp^T[half, j-chunk] [64,128], rhs = Qp^T [64, i] ->
    S^T [128 j, i] in PSUM; the two K=64 matmuls of a j-chunk pair run
    CONCURRENTLY in PE row-groups 0:64 / 64:128 (tile_position row tiling);
    exp fused with the 1/8 scale on ScalarE (no max-subtraction:
    |scores/8| <= ~3 so fp32 exp is safe)
  - PV: lhsT = v[j-chunk] [128, 66], rhs = P^T [128, i], accumulated over j
    in PSUM -> out^T [66, i] (row 64 = denominator)
  - epilogue: PE-transpose out^T, reciprocal + scale on DVE, DMA out.
"""

import sys

if "/opt/trn_rl_repo" not in sys.path:
    sys.path.insert(0, "/opt/trn_rl_repo")

from contextlib import ExitStack

import numpy as np

import concourse.bass as bass
import concourse.tile as tile
from concourse import bacc, mybir
from concourse.bass_utils import run_bass_kernel_spmd
from concourse.masks import make_identity

F32 = mybir.dt.float32
import os as _os
XT_BUFS = int(_os.environ.get("XT_BUFS", "2"))
ST_W = int(_os.environ.get("ST_W", "1024"))
ST_BUFS = int(_os.environ.get("ST_BUFS", "2"))
F32R = mybir.dt.float32r
B, S, C, D = 4, 4096, 512, 64
D2 = D + 2          # v padded with [ones, zeros] cols (fp32r needs even sizes)
N_CORES = 8
SQ = S // 2          # query rows per core
NJ = S // 128        # 32 key chunks of 128 rows
IH = SQ // 2         # 1024: i-half processed per PSUM residency
EXP = mybir.ActivationFunctionType.Exp
COPY = mybir.ActivationFunctionType.Copy

_CACHE = {}


def _emit(nc, tc, aps):
    q_d, k_d, v_d, wq_d, wk_d, wvp_d, bq_d, bk_d, bvp_d, out_d = aps

    ctx = ExitStack()
    const = ctx.enter_context(tc.tile_pool(name="const", bufs=1))
    persist = ctx.enter_context(tc.tile_pool(name="persist", bufs=1))
    stage_p = ctx.enter_context(tc.tile_pool(name="stage", bufs=4))
    xts_p = ctx.enter_context(tc.tile_pool(name="xts", bufs=3))
    pt_p = ctx.enter_context(tc.tile_pool(name="pt", bufs=3))
    ep_p = ctx.enter_context(tc.tile_pool(name="ep", bufs=2))
    small_p = ctx.enter_context(tc.tile_pool(name="small", bufs=4))
    out_p = ctx.enter_context(tc.tile_pool(name="outp", bufs=2))
    # PSUM budget (8 banks): shared xt/pp pool 2 + st 2x2 + po 2 = 8
    xt_ps = ctx.enter_context(tc.tile_pool(name="xtps", bufs=XT_BUFS, space="PSUM"))
    st_ps = ctx.enter_context(tc.tile_pool(name="stps", bufs=ST_BUFS, space="PSUM"))
    po_ps = ctx.enter_context(tc.tile_pool(name="pops", bufs=1, space="PSUM"))

    ident32 = const.tile([128, 128], F32)
    make_identity(nc, ident32[:])
    ident = const.tile([128, 128], F32R)
    nc.vector.tensor_copy(ident[:], ident32[:])
    wq_sb = const.tile([128, 4, D], F32R)
    nc.gpsimd.dma_start(wq_sb[:], wq_d.rearrange("(cc p) d -> p cc d", p=128))
    wk_sb = const.tile([128, 4, D], F32R)
    nc.gpsimd.dma_start(wk_sb[:], wk_d.rearrange("(cc p) d -> p cc d", p=128))
    wvp_sb = const.tile([128, 4, D2], F32R)
    nc.gpsimd.dma_start(wvp_sb[:], wvp_d.rearrange("(cc p) d -> p cc d", p=128))
    bq2_sb = const.tile([128, 1], F32)
    nc.sync.dma_start(bq2_sb[:D, :], bq_d[:])
    nc.sync.dma_start(bq2_sb[D:, :], bq_d[:])
    bk2_sb = const.tile([128, 1], F32)
    nc.sync.dma_start(bk2_sb[:D, :], bk_d[:])
    nc.sync.dma_start(bk2_sb[D:, :], bk_d[:])
    bvp_sb = const.tile([D2, 1], F32)
    nc.sync.dma_start(bvp_sb[:], bvp_d[:])

    qpt = persist.tile([128, SQ], F32R)          # Qp^T duplicated on both halves
    kpt = persist.tile([128, S // 2], F32R)      # Kp^T dual-half (even|odd chunks)
    v_sb = persist.tile([128, NJ, D2], F32R)     # v natural + ones col

    def copy_dve(dst, src):
        nc.vector.tensor_copy(dst, src)

    def copy_act(dst, src):
        nc.scalar.activation(dst, src, COPY)

    def prework(x_ap, g, w_sb, m, sink, copy):
        """Process 512 rows (group g) of one activation: load, transpose,
        project; sink(pp) consumes the [m, 512] projected PSUM tile."""
        stg = stage_p.tile([128, 4, 512], F32R, tag="stage")
        nc.gpsimd.dma_start(
            stg[:], x_ap[g * 512 : (g + 1) * 512, :].rearrange("(r p) c -> p r c", p=128)
        )
        xts = xts_p.tile([128, 4, 512], F32R, tag="xts")
        for cc in range(4):
            xtp = xt_ps.tile([128, 512], F32R, tag="xt")
            for r in range(4):
                nc.tensor.transpose(
                    xtp[:, r * 128 : (r + 1) * 128],
                    stg[:, r, cc * 128 : (cc + 1) * 128],
                    ident[:],
                )
            copy(xts[:, cc, :], xtp[:])
        pp = xt_ps.tile([D2, 512], F32, tag="xt")
        for cc in range(4):
            nc.tensor.matmul(
                pp[:m, :], w_sb[:, cc, :m], xts[:, cc, :],
                start=(cc == 0), stop=(cc == 3),
            )
        sink(pp)

    def sink_q(g):
        def f(pp):
            sl = slice(g * 512, (g + 1) * 512)
            nc.vector.tensor_scalar_add(qpt[:D, sl], pp[:D, :], bq2_sb[:D, :])
            nc.sync.dma_start(qpt[D:, sl], qpt[:D, sl])
        return f

    def sink_k(g):
        # pp [64, 512] = j-chunks 4g..4g+3; even chunks -> kpt[0:64],
        # odd chunks -> partition-shift DMA -> kpt[64:128]
        def f(pp):
            ppv = pp[:D, :].rearrange("p (b n) -> p b n", n=128)
            dst = kpt[:D, g * 256 : (g + 1) * 256].rearrange("p (b n) -> p b n", n=128)
            nc.vector.tensor_scalar_add(dst, ppv[:, 0::2, :], bk2_sb[:D, :])
            ktmp = ep_p.tile([D, 256], F32R, tag="ktmp")
            nc.vector.tensor_scalar_add(
                ktmp[:].rearrange("p (b n) -> p b n", n=128),
                ppv[:, 1::2, :],
                bk2_sb[:D, :],
            )
            nc.sync.dma_start(kpt[D:, g * 256 : (g + 1) * 256], ktmp[:])
        return f

    def sink_v(g):
        def f(pp):
            vt = ep_p.tile([D2, 512], F32R, tag="vt")
            nc.vector.tensor_scalar_add(vt[:], pp[:, :], bvp_sb[:])
            for r in range(4):
                vnp = xt_ps.tile([128, D2], F32R, tag="xt")
                nc.tensor.transpose(
                    vnp[:], vt[:, r * 128 : (r + 1) * 128], ident[:D2, :D2]
                )
                copy_dve(v_sb[:, g * 4 + r, :], vnp[:])
        return f

    # query prework: 4 groups
    for g in range(SQ // 512):
        prework(q_d, g, wq_sb, D, sink_q(g), copy_dve)

    def attention_jp(jp, ih, po, first, last):
        # j-pair: even chunk 2*jp (kpt rows 0:64), odd 2*jp+1 (rows 64:128)
        for w in range(IH // ST_W):
            sts = []
            for half in range(2):
                st = st_ps.tile([128, ST_W], F32, tag="st")
                for n in range(ST_W // 512):
                    nc.tensor.matmul(
                        st[:, n * 512 : (n + 1) * 512],
                        kpt[half * D : (half + 1) * D, jp * 128 : (jp + 1) * 128],
                        qpt[half * D : (half + 1) * D,
                            ih * IH + w * ST_W + n * 512 : ih * IH + w * ST_W + (n + 1) * 512],
                        tile_position=(half * D, 0),
                    )
                pt = pt_p.tile([128, ST_W], F32R, tag="pt")
                nc.scalar.activation(pt[:], st[:], EXP, scale=0.125)
                sts.append(pt)
            for half in range(2):
                for n in range(ST_W // 512):
                    nc.tensor.matmul(
                        po[:, w * ST_W + n * 512 : w * ST_W + (n + 1) * 512],
                        v_sb[:, 2 * jp + half, :],
                        sts[half][:, n * 512 : (n + 1) * 512],
                        start=(first and half == 0), stop=(last and half == 1),
                    )

    def epilogue(ih, po):
        ot = ep_p.tile([D2, IH], F32R, tag="ot")
        nc.vector.tensor_copy(ot[:], po[:])
        osb = out_p.tile([128, IH // 128, D], F32, tag="osb")
        for t in range(IH // 128):
            onat = xt_ps.tile([128, D2], F32R, tag="xt")
            nc.tensor.transpose(
                onat[:], ot[:, t * 128 : (t + 1) * 128], ident[:D2, :D2]
            )
            rs = small_p.tile([128, 1], F32, tag="rs")
            nc.vector.reciprocal(rs[:], onat[:, D : D + 1])
            nc.vector.tensor_scalar_mul(osb[:, t, :], onat[:, :D], rs[:])
        nc.sync.dma_start(
            out_d[ih * IH : (ih + 1) * IH, :].rearrange("(t p) d -> p t d", p=128),
            osb[:],
        )

    # interleave k/v prework with first i-half attention
    po0 = po_ps.tile([D2, IH], F32, tag="po")
    for jg in range(S // 512):
        prework(k_d, jg, wk_sb, D, sink_k(jg), copy_dve)
        prework(v_d, jg, wvp_sb, D2, sink_v(jg), copy_dve)
        for jp in range(jg * 2, jg * 2 + 2):
            attention_jp(jp, 0, po0, first=(jp == 0), last=(jp == NJ // 2 - 1))
    epilogue(0, po0)

    po1 = po_ps.tile([D2, IH], F32, tag="po")
    for jp in range(NJ // 2):
        attention_jp(jp, 1, po1, first=(jp == 0), last=(jp == NJ // 2 - 1))
    epilogue(1, po1)
    ctx.close()


def _build(reps=1):
    nc = bacc.Bacc("TRN2", target_bir_lowering=False, debug=False, num_devices=N_CORES)
    aps = (
        nc.dram_tensor("q", [SQ, C], F32, kind="ExternalInput").ap(),
        nc.dram_tensor("k", [S, C], F32, kind="ExternalInput").ap(),
        nc.dram_tensor("v", [S, C], F32, kind="ExternalInput").ap(),
        nc.dram_tensor("wq", [C, D], F32, kind="ExternalInput").ap(),
        nc.dram_tensor("wk", [C, D], F32, kind="ExternalInput").ap(),
        nc.dram_tensor("wvp", [C, D2], F32, kind="ExternalInput").ap(),
        nc.dram_tensor("bq", [D, 1], F32, kind="ExternalInput").ap(),
        nc.dram_tensor("bk", [D, 1], F32, kind="ExternalInput").ap(),
        nc.dram_tensor("bvp", [D2, 1], F32, kind="ExternalInput").ap(),
        nc.dram_tensor("out", [SQ, D], F32, kind="ExternalOutput").ap(),
    )
    with tile.TileContext(nc) as tc:
        for _ in range(reps):
            _emit(nc, tc, aps)
    nc.compile()
    return nc


def get_nc():
    if "nc" not in _CACHE:
        _CACHE["nc"] = _build()
    return _CACHE["nc"]


def make_in_maps(query, key_, value, Wq, bq, Wk, bk, Wv, bv):
    query, key_, value, Wq, bq, Wk, bk, Wv, bv = (
        np.asarray(a, dtype=np.float32)
        for a in (query, key_, value, Wq, bq, Wk, bk, Wv, bv)
    )
    wvp = np.concatenate([Wv, np.zeros((C, 2), np.float32)], axis=1)
    bvp = np.concatenate([bv, np.asarray([1.0, 0.0], np.float32)])[:, None]
    shared = {
        "wq": np.ascontiguousarray(Wq),
        "wk": np.ascontiguousarray(Wk),
        "wvp": np.ascontiguousarray(wvp),
        "bq": np.ascontiguousarray(bq[:, None]),
        "bk": np.ascontiguousarray(bk[:, None]),
        "bvp": np.ascontiguousarray(bvp),
    }
    in_maps = []
    for c in range(N_CORES):
        b, h = divmod(c, 2)
        in_maps.append(
            {
                "q": np.ascontiguousarray(query[b, h * SQ : (h + 1) * SQ, :]),
                "k": np.ascontiguousarray(key_[b]),
                "v": np.ascontiguousarray(value[b]),
                **shared,
            }
        )
    return in_maps


def assemble(results):
    out = np.empty((B, S, D), np.float32)
    for c in range(N_CORES):
        b, h = divmod(c, 2)
        out[b, h * SQ : (h + 1) * SQ, :] = results[c]["out"]
    return out


def kernel(query, key_, value, Wq, bq, Wk, bk, Wv, bv, **_):
    nc = get_nc()
    in_maps = make_in_maps(query, key_, value, Wq, bq, Wk, bk, Wv, bv)
    res = run_bass_kernel_spmd(nc, in_maps, list(range(N_CORES)))
    return assemble(res.results)
